# revision 4
# baseline (speedup 1.0000x reference)
"""BlockReLU Trainium2 kernel (8-core data-parallel over batch).

Reference semantics (per [N, C, H, W] f32 input):
  channels  0:16  block (1,1): out = x * (x > 0)            == relu(x)
  channels 16:32  block (2,2): out = x * (mean_2x2(x) > 0)
  channels 32:48  block (4,4): out = x * (mean_4x4(x) > 0)
  channels 48:56  block (8,8): out = x * (mean_8x8(x) > 0)
  channels 56:64  identity

sign(mean) == sign(sum) (the divisor is a power of two), so block sums
are used instead of means.

Per-core layout: the batch shard [2, 64, 192, 192] is host-permuted so
each (channel, n) image sits on one SBUF partition (free dim = flattened
H*W), with channel groups in partition ranges chosen to satisfy the BIR
partition-window rule (base % 32 == 0; >32-partition windows at base 0):

  partitions  0:32   block (2,2) channels (c 16:32)
  partitions 32:64   block (4,4) channels (c 32:48)
  partitions 64:80   block (8,8) channels (c 48:56)
  partitions 80:96   identity    channels (c 56:64)  (no compute)
  partitions 96:128  block (1,1) channels (c  0:16)  (relu on ScalarE)

The image is processed in row-chunks of R rows:
  - 2x2 block sums: two chained pairwise adds (DVE tensor_tensor) on
    partitions [0:80] at once; 4x4 sums from 2x2 sums on [32:64] and
    [64:80]; 8x8 from 4x4 on [64:80].
  - masks = relu(sign(sum)) on ScalarE (keeps DVE tensor_scalar 2-port
    perf mode off the shared SBUF port so GpSimd never stalls).
  - masked multiply = broadcast tensor_tensor, one sub-op per block-row
    offset dh (keeps APs at <=3 free dims), split between DVE and GpSimd.
  - DMA via HWDGE (nc.sync) never contends with compute.
"""

import json
import re

import numpy as np

N, C, H, W = 16, 64, 192, 192
NCORES = 8
NB = N // NCORES  # batch per core
HW = H * W

R = 24  # rows per chunk (multiple of 8)
L = R * W
NCHUNK = H // R

XT_BUFS = 3
TMP_BUFS = 2

# partition-group channel order (host-side permutation)
PERM = (
    list(range(16, 32))
    + list(range(32, 48))
    + list(range(48, 56))
    + list(range(56, 64))
    + list(range(0, 16))
)
IPERM = np.argsort(np.array(PERM))

# which dh sub-ops of each group's masked multiply run on DVE (rest GpSimd)
G2_DVE_DH = (0, 1)  # of 2
G4_DVE_DH = (0, 1)  # of 4
G8_DVE_DH = ()  # of 8

_CACHE = {}


def _split_multi_waits(bir_json: bytes) -> bytes:
    """This walrus build rejects >1 sync-wait per instruction; hoist extra
    waits onto fresh single-wait NoOps on the same engine."""
    m = json.loads(bir_json)
    max_idx = 0
    for f in m.get("functions", []):
        for b in f.get("blocks", []):
            for ins in b.get("instructions", []):
                mt = re.match(r"I-(\d+)$", ins.get("name", ""))
                if mt:
                    max_idx = max(max_idx, int(mt.group(1)))
    next_idx = max_idx + 1
    for f in m.get("functions", []):
        for b in f.get("blocks", []):
            out = []
            for ins in b.get("instructions", []):
                si = ins.get("sync_info")
                waits = (si or {}).get("on_wait") or []
                if len(waits) > 1:
                    for w in waits[:-1]:
                        out.append(
                            {
                                "debug": ins.get("debug"),
                                "engine": ins["engine"],
                                "ins": [],
                                "name": f"I-{next_idx}",
                                "opcode": "NoOp",
                                "outs": [],
                                "sync_info": {"on_wait": [w], "on_update": []},
                            }
                        )
                        next_idx += 1
                    si["on_wait"] = [waits[-1]]
                out.append(ins)
            b["instructions"] = out
    return json.dumps(m).encode()


def _install_birpatch():
    import concourse.bass2jax as b2j
    import concourse.bass_utils as bu

    if getattr(bu, "_split_waits_installed", False):
        return
    orig = bu.compile_bir_kernel

    def compile_bir_kernel_split(bir_json, tmpdir, neff_name="file.neff"):
        return orig(_split_multi_waits(bir_json), tmpdir, neff_name)

    bu.compile_bir_kernel = compile_bir_kernel_split
    b2j.compile_bir_kernel = compile_bir_kernel_split
    bu._split_waits_installed = True


def _build_nc():
    import concourse.bass as bass
    import concourse.mybir as mybir
    from concourse.tile import TileContext

    _install_birpatch()

    f32 = mybir.dt.float32
    ALU = mybir.AluOpType
    AF = mybir.ActivationFunctionType

    nc = bass.Bass("TRN2", debug=False)
    # host passes the shard pre-permuted/transposed to [(c n), h*w] = [128, HW]
    xs = nc.dram_tensor("x", [C * NB, HW], f32, kind="ExternalInput").ap()
    ys = nc.dram_tensor("y", [C * NB, HW], f32, kind="ExternalOutput").ap()

    def vadd(out, in0, in1):
        nc.vector.tensor_tensor(out=out, in0=in0, in1=in1, op=ALU.add)

    def pool2x2(src, dst, tmp, p0, p1, w):
        """dst[p0:p1] <- 2x2 block sums of src[p0:p1] ([rows, w] per part)."""
        v = src[p0:p1, :].rearrange("p (r a t) -> p r a t", a=w // 2, t=2)
        vadd(
            tmp[p0:p1, :].rearrange("p (r a) -> p r a", a=w // 2),
            v[:, :, :, 0],
            v[:, :, :, 1],
        )
        u = tmp[p0:p1, :].rearrange("p (r t a) -> p r t a", t=2, a=w // 2)
        vadd(
            dst[p0:p1, :].rearrange("p (r a) -> p r a", a=w // 2),
            u[:, :, 0, :],
            u[:, :, 1, :],
        )

    with TileContext(nc) as tc:
        with (
            tc.tile_pool(name="xt", bufs=XT_BUFS) as px,
            tc.tile_pool(name="tmp", bufs=TMP_BUFS) as pt,
        ):
            for ck in range(NCHUNK):
                xt = px.tile([128, L], f32, tag="xt")
                t1 = pt.tile([80, L // 2], f32, tag="t1")
                sa = pt.tile([80, L // 4], f32, tag="sa")
                t2 = pt.tile([80, L // 8], f32, tag="t2")
                sb = pt.tile([80, L // 16], f32, tag="sb")
                t3 = pt.tile([80, L // 32], f32, tag="t3")
                sc = pt.tile([80, L // 64], f32, tag="sc")

                nc.sync.dma_start(out=xt[:, :], in_=xs[:, ck * L : (ck + 1) * L])

                # block sums: 2x2 on [0:80]; 4x4 on [32:64] + [64:80]; 8x8 on [64:80]
                pool2x2(xt, sa, t1, 0, 80, W)
                pool2x2(sa, sb, t2, 32, 64, W // 2)
                pool2x2(sa, sb, t2, 64, 80, W // 2)
                pool2x2(sb, sc, t3, 64, 80, W // 4)

                # masks: relu(sign(sum)) in place, on ScalarE
                for m_ap in (sa[0:32, :], sb[32:64, :], sc[64:80, :]):
                    nc.scalar.activation(out=m_ap, in_=m_ap, func=AF.Sign)
                    nc.scalar.activation(out=m_ap, in_=m_ap, func=AF.Relu)

                # block (1,1): relu on ScalarE
                nc.scalar.activation(out=xt[96:128, :], in_=xt[96:128, :], func=AF.Relu)

                # masked multiplies (one sub-op per block-row offset dh)
                def mult(p0, p1, bs, mask_tile, dve_dhs):
                    npart = p1 - p0
                    a = R // bs  # block-rows in chunk
                    b = W // bs  # block-cols
                    vx = xt[p0:p1, :].rearrange(
                        "p (a t b c) -> p a t b c", t=bs, b=b, c=bs
                    )
                    m = (
                        mask_tile[p0:p1, :]
                        .rearrange("p (a b) -> p a b", b=b)
                        .unsqueeze(3)
                        .broadcast_to([npart, a, b, bs])
                    )
                    for dh in range(bs):
                        eng = nc.vector if dh in dve_dhs else nc.gpsimd
                        o = vx[:, :, dh, :, :]
                        eng.tensor_tensor(out=o, in0=o, in1=m, op=ALU.mult)

                mult(0, 32, 2, sa, G2_DVE_DH)
                mult(32, 64, 4, sb, G4_DVE_DH)
                mult(64, 80, 8, sc, G8_DVE_DH)

                nc.sync.dma_start(out=ys[:, ck * L : (ck + 1) * L], in_=xt[:, :])

    return nc


def kernel(activation: np.ndarray) -> np.ndarray:
    from concourse import bass_utils

    if "nc" not in _CACHE:
        _CACHE["nc"] = _build_nc()
    nc = _CACHE["nc"]

    in_maps = [
        {
            "x": np.ascontiguousarray(
                activation[k * NB : (k + 1) * NB][:, PERM].transpose(1, 0, 2, 3)
            ).reshape(C * NB, HW)
        }
        for k in range(NCORES)
    ]
    res = bass_utils.run_bass_kernel_spmd(nc, in_maps, core_ids=list(range(NCORES)))
    out = np.empty((N, C, H, W), dtype=activation.dtype)
    for k in range(NCORES):
        yk = res.results[k]["y"].reshape(C, NB, H, W).transpose(1, 0, 2, 3)
        out[k * NB : (k + 1) * NB] = yk[:, IPERM]
    return out


# revision 5
# speedup vs baseline: 1.3107x; 1.3107x over previous
"""BlockReLU Trainium2 kernel (8-core data-parallel over batch).

Reference semantics (per [N, C, H, W] f32 input):
  channels  0:16  block (1,1): out = x * (x > 0)            == relu(x)
  channels 16:32  block (2,2): out = x * (mean_2x2(x) > 0)
  channels 32:48  block (4,4): out = x * (mean_4x4(x) > 0)
  channels 48:56  block (8,8): out = x * (mean_8x8(x) > 0)
  channels 56:64  identity

sign(mean) == sign(sum) (the divisor is a power of two), so block sums
are used instead of means.

Per-core layout: the batch shard [2, 64, 192, 192] is host-permuted so
each (channel, n) image sits on one SBUF partition (free dim = flattened
H*W), with channel groups in partition ranges chosen to satisfy the BIR
partition-window rule (base % 32 == 0; >32-partition windows at base 0):

  partitions  0:32   block (2,2) channels (c 16:32)
  partitions 32:64   block (4,4) channels (c 32:48)
  partitions 64:80   block (8,8) channels (c 48:56)
  partitions 80:96   identity    channels (c 56:64)  (no compute)
  partitions 96:128  block (1,1) channels (c  0:16)  (relu on ScalarE)

The image is processed in row-chunks of R rows:
  - 2x2 block sums: two chained pairwise adds (DVE tensor_tensor) on
    partitions [0:80] at once; 4x4 sums from 2x2 sums on [32:64] and
    [64:80]; 8x8 from 4x4 on [64:80].
  - masks = relu(sign(sum)) on ScalarE (keeps DVE tensor_scalar 2-port
    perf mode off the shared SBUF port so GpSimd never stalls).
  - masked multiply = broadcast tensor_tensor, one sub-op per block-row
    offset dh (keeps APs at <=3 free dims), split between DVE and GpSimd.
  - DMA via HWDGE (nc.sync) never contends with compute.
"""

import json
import re

import numpy as np

N, C, H, W = 16, 64, 192, 192
NCORES = 8
NB = N // NCORES  # batch per core
HW = H * W

R = 24  # rows per chunk (multiple of 8)
L = R * W
NCHUNK = H // R

XT_BUFS = 3
TMP_BUFS = 2
PSUM_BUFS = 2

# partition-group channel order (host-side permutation)
PERM = (
    list(range(16, 32))
    + list(range(32, 48))
    + list(range(48, 56))
    + list(range(56, 64))
    + list(range(0, 16))
)
IPERM = np.argsort(np.array(PERM))

# which dh sub-ops of each group's masked multiply run on DVE (rest GpSimd)
G2_DVE_DH = (0, 1)  # of 2
G4_DVE_DH = (0, 1)  # of 4
G8_DVE_DH = ()  # of 8

_CACHE = {}


def _split_multi_waits(bir_json: bytes) -> bytes:
    """This walrus build rejects >1 sync-wait per instruction; hoist extra
    waits onto fresh single-wait NoOps on the same engine."""
    m = json.loads(bir_json)
    max_idx = 0
    for f in m.get("functions", []):
        for b in f.get("blocks", []):
            for ins in b.get("instructions", []):
                mt = re.match(r"I-(\d+)$", ins.get("name", ""))
                if mt:
                    max_idx = max(max_idx, int(mt.group(1)))
    next_idx = max_idx + 1
    for f in m.get("functions", []):
        for b in f.get("blocks", []):
            out = []
            for ins in b.get("instructions", []):
                si = ins.get("sync_info")
                waits = (si or {}).get("on_wait") or []
                if len(waits) > 1:
                    for w in waits[:-1]:
                        out.append(
                            {
                                "debug": ins.get("debug"),
                                "engine": ins["engine"],
                                "ins": [],
                                "name": f"I-{next_idx}",
                                "opcode": "NoOp",
                                "outs": [],
                                "sync_info": {"on_wait": [w], "on_update": []},
                            }
                        )
                        next_idx += 1
                    si["on_wait"] = [waits[-1]]
                out.append(ins)
            b["instructions"] = out
    return json.dumps(m).encode()


def _install_birpatch():
    import concourse.bass2jax as b2j
    import concourse.bass_utils as bu

    if getattr(bu, "_split_waits_installed", False):
        return
    orig = bu.compile_bir_kernel

    def compile_bir_kernel_split(bir_json, tmpdir, neff_name="file.neff"):
        return orig(_split_multi_waits(bir_json), tmpdir, neff_name)

    bu.compile_bir_kernel = compile_bir_kernel_split
    b2j.compile_bir_kernel = compile_bir_kernel_split
    bu._split_waits_installed = True


def _build_nc():
    import concourse.bass as bass
    import concourse.mybir as mybir
    from concourse.tile import TileContext

    _install_birpatch()

    f32 = mybir.dt.float32
    ALU = mybir.AluOpType
    AF = mybir.ActivationFunctionType
    AX = mybir.AxisListType

    nc = bass.Bass("TRN2", debug=False)
    # host passes the shard pre-permuted/transposed to [(c n), h*w] = [128, HW]
    xs = nc.dram_tensor("x", [C * NB, HW], f32, kind="ExternalInput").ap()
    ys = nc.dram_tensor("y", [C * NB, HW], f32, kind="ExternalOutput").ap()

    def pool_reduce(eng, src, dst, p0, p1, w):
        """dst[p0:p1] <- 2x2 block sums of src[p0:p1] ([rows, w] per part)."""
        v = src[p0:p1, :].rearrange("p (r t a c) -> p r t a c", t=2, a=w // 2, c=2)
        eng.tensor_reduce(
            out=dst[p0:p1, :].rearrange("p (r a) -> p r a", a=w // 2),
            in_=v.transpose([0, 1, 3, 2, 4]),
            axis=AX.XY,
            op=ALU.add,
        )

    with TileContext(nc) as tc:
        with (
            tc.tile_pool(name="xt", bufs=XT_BUFS) as px,
            tc.tile_pool(name="tmp", bufs=TMP_BUFS) as pt,
            tc.tile_pool(name="ps", bufs=PSUM_BUFS, space="PSUM") as pp,
        ):
            for ck in range(NCHUNK):
                xt = px.tile([128, L], f32, tag="xt")
                sa = pp.tile([80, L // 4], f32, tag="sa")
                sb = pp.tile([80, L // 16], f32, tag="sb")
                sb_sb = pt.tile([64, L // 16], f32, tag="sb_sb")
                sc = pt.tile([80, L // 64], f32, tag="sc")

                nc.sync.dma_start(out=xt[:, :], in_=xs[:, ck * L : (ck + 1) * L])

                # block sums (DVE tensor_reduce: single-port, no GpSimd contention)
                pool_reduce(nc.vector, xt, sa, 0, 80, W)       # 2x2 -> PSUM
                pool_reduce(nc.vector, sa, sb, 32, 64, W // 2) # 4x4 (g4) -> PSUM
                pool_reduce(nc.vector, sa, sb, 64, 80, W // 2) # 4x4 (g8) -> PSUM
                pool_reduce(nc.vector, sb, sc, 64, 80, W // 4) # 8x8 (g8) -> SBUF

                # masks: relu(sign(sum)) in place, on ScalarE
                for m_ap in (sa[0:32, :], sb[32:64, :], sc[64:80, :]):
                    nc.scalar.activation(out=m_ap, in_=m_ap, func=AF.Sign)
                    nc.scalar.activation(out=m_ap, in_=m_ap, func=AF.Relu)
                # SBUF copy of the g4 mask for GpSimd's share
                nc.scalar.copy(out=sb_sb[32:64, :], in_=sb[32:64, :])

                # block (1,1): relu on ScalarE
                nc.scalar.activation(out=xt[96:128, :], in_=xt[96:128, :], func=AF.Relu)

                # masked multiplies (one sub-op per block-row offset dh)
                def mult(p0, p1, bs, mask_dve, mask_gp, dve_dhs):
                    npart = p1 - p0
                    a = R // bs
                    b = W // bs
                    vx = xt[p0:p1, :].rearrange(
                        "p (a t b c) -> p a t b c", t=bs, b=b, c=bs
                    )

                    def bc(mt):
                        return (
                            mt[p0:p1, :]
                            .rearrange("p (a b) -> p a b", b=b)
                            .unsqueeze(3)
                            .broadcast_to([npart, a, b, bs])
                        )

                    for dh in range(bs):
                        o = vx[:, :, dh, :, :]
                        if dh in dve_dhs:
                            nc.vector.tensor_tensor(out=o, in0=o, in1=bc(mask_dve), op=ALU.mult)
                        else:
                            nc.gpsimd.tensor_tensor(out=o, in0=o, in1=bc(mask_gp), op=ALU.mult)

                mult(0, 32, 2, sa, None, G2_DVE_DH)
                mult(32, 64, 4, sb, sb_sb, G4_DVE_DH)
                mult(64, 80, 8, sc, sc, G8_DVE_DH)

                nc.sync.dma_start(out=ys[:, ck * L : (ck + 1) * L], in_=xt[:, :])

    return nc


def kernel(activation: np.ndarray) -> np.ndarray:
    from concourse import bass_utils

    if "nc" not in _CACHE:
        _CACHE["nc"] = _build_nc()
    nc = _CACHE["nc"]

    in_maps = [
        {
            "x": np.ascontiguousarray(
                activation[k * NB : (k + 1) * NB][:, PERM].transpose(1, 0, 2, 3)
            ).reshape(C * NB, HW)
        }
        for k in range(NCORES)
    ]
    res = bass_utils.run_bass_kernel_spmd(nc, in_maps, core_ids=list(range(NCORES)))
    out = np.empty((N, C, H, W), dtype=activation.dtype)
    for k in range(NCORES):
        yk = res.results[k]["y"].reshape(C, NB, H, W).transpose(1, 0, 2, 3)
        out[k * NB : (k + 1) * NB] = yk[:, IPERM]
    return out
